# revision 1
# baseline (speedup 1.0000x reference)
"""Trainium2 Bass kernel for nn_Encoder_LSTM (4x LSTMCell with zero state over
packed ragged tokens).

Math (from the reference): all rows independent; for each output row j with
source row s(j) (the ragged gather), and each of 4 layers:
    gates = x @ W_ih^T + (b_ih + b_hh);  i, f, g, o = split(gates)
    c = sigmoid(i) * tanh(g);  h = sigmoid(o) * tanh(c)      (f is unused)
Outputs: (output=h4, h1, c1, h2, c2, h3, c3, h4, c4), each [sum(bs), 512] fp32.

Strategy:
  - Only U = max_i (i+1)*bs_i source rows are distinct (the ragged gather
    re-reads rows); compute each distinct row once and write it to every
    duplicate output position.
  - Shard distinct source rows round-robin (r mod 8 -> core): all 8 cores get
    identical tile counts and near-identical output-row counts. Within a
    core, order rows by duplicate-count (coverage) DESCENDING. Then for a
    128-row tile t and duplicate slot k, the rows having a k-th duplicate are
    exactly a PREFIX p < m[t,k] of the tile -- so every store is a plain
    contiguous DMA (no indirect/scatter DMAs at all). The device output slab
    is ordered by (tile, slot, prefix); the host maps slab rows back to
    logical output rows with a precomputed permutation.
  - All 8 per-layer outputs live in one fused DRAM tensor (output j occupies
    rows [j*O_alloc, (j+1)*O_alloc)); each tile keeps h/c of all 4 layers in
    one [128, 8*512] SBUF tile.
  - Per 128-row tile: PE-transpose activations into [feat, tok] chunks, then
    3-gate matmul (bf16, K=512 contraction in 4 chunks, N=512 chunks) into
    PSUM, bias-add on DVE, sigmoid/tanh on ACT (one table set), muls on DVE.
  - Host only: input layout prep (weight transpose/packing, row gather of
    packed_x) and output slab reassembly. All math and all output bytes are
    produced on-device.
"""

import sys

if "/opt/trn_rl_repo" not in sys.path:
    sys.path.insert(0, "/opt/trn_rl_repo")

import numpy as np
import ml_dtypes

P = 128
H = 512
G = 1536          # 3 packed gates [i, o, g] * 512
J = 8             # fused outputs [h1, c1, h2, c2, h3, c3, h4, c4]
NCORES = 8
OUT_NAMES = ["h1", "c1", "h2", "c2", "h3", "c3", "h4", "c4"]


# ---------------------------------------------------------------- host plan

def _make_plan(batch_sizes):
    bs = np.asarray(batch_sizes).astype(np.int64)
    s = np.concatenate([i * b + np.arange(b) for i, b in enumerate(bs)]).astype(np.int64)
    Nout = int(s.size)
    U = int(s.max()) + 1
    cov = np.bincount(s, minlength=U)

    cores = []
    for c in range(NCORES):
        src = np.arange(c, U, NCORES, dtype=np.int64)
        order = np.argsort(-cov[src], kind="stable")
        src_o = src[order]                       # this core's rows, cov-desc
        out_js = np.flatnonzero((s % NCORES) == c)   # global out rows (asc)
        src_of_slab = s[out_js]
        sort_slab = np.argsort(src_of_slab, kind="stable")
        srcs_sorted = src_of_slab[sort_slab]
        lo = np.searchsorted(srcs_sorted, src_o, "left")
        hi = np.searchsorted(srcs_sorted, src_o, "right")
        cores.append(dict(src_o=src_o, out_js=out_js, sort_slab=sort_slab,
                          lo=lo, hi=hi, covs=(hi - lo)))

    n_src = [len(cc["src_o"]) for cc in cores]
    T_tiles = max((n + P - 1) // P for n in n_src)

    # per-(tile, slot) prefix length, uniform across cores (max)
    MT = []          # list over t of list over k of m[t][k]
    for t in range(T_tiles):
        K = 1
        for cc in cores:
            cv = cc["covs"][t * P:(t + 1) * P]
            if len(cv):
                K = max(K, int(cv.max()))
        ms = []
        for k in range(K):
            m = 0
            for cc in cores:
                cv = cc["covs"][t * P:(t + 1) * P]
                if len(cv):
                    m = max(m, int((cv > k).sum()))
            ms.append(m)
        MT.append(ms)
    # slab layout: block (t,k) occupies rows [B[t][k], B[t][k]+m)
    B = []
    off = 0
    for ms in MT:
        Bs_ = []
        for m in ms:
            Bs_.append(off)
            off += m
        B.append(Bs_)
    O_alloc = off
    OBIG = J * O_alloc

    # per-core slab -> logical-output-row permutation
    for cc in cores:
        slab_rows, glob_rows = [], []
        covs, lo, sort_slab = cc["covs"], cc["lo"], cc["sort_slab"]
        out_js = cc["out_js"]
        n = len(cc["src_o"])
        for t in range(T_tiles):
            for k, m in enumerate(MT[t]):
                base = B[t][k]
                for p in range(m):
                    r = t * P + p
                    if r < n and covs[r] > k:
                        slab_rows.append(base + p)
                        glob_rows.append(out_js[sort_slab[lo[r] + k]])
        cc["slab_rows"] = np.asarray(slab_rows, np.int64)
        cc["glob_rows"] = np.asarray(glob_rows, np.int64)

    return dict(s=s, Nout=Nout, U=U, cores=cores, T_tiles=T_tiles,
                MT=MT, B=B, O_alloc=O_alloc, OBIG=OBIG)


def _pack_weights(inputs):
    """-> w [4,4,P,G] bf16 (W^T igo, 4 k-chunks per layer), b [4,P,G] f32."""
    w = np.zeros((4, 4, P, G), ml_dtypes.bfloat16)
    b = np.zeros((4, P, G), np.float32)
    for li in range(4):
        W = np.asarray(inputs[f"W_ih{li+1}"], np.float32)        # [2048, 512]
        bb = (np.asarray(inputs[f"b_ih{li+1}"], np.float32)
              + np.asarray(inputs[f"b_hh{li+1}"], np.float32))   # [2048]
        Wigo = np.concatenate([W[0:512], W[1536:2048], W[1024:1536]], axis=0)
        bigo = np.concatenate([bb[0:512], bb[1536:2048], bb[1024:1536]])
        WT = np.ascontiguousarray(Wigo.T)                        # [512, 1536]
        for k in range(4):
            w[li, k] = WT[k * P:(k + 1) * P].astype(ml_dtypes.bfloat16)
        b[li] = np.broadcast_to(bigo[None, :], (P, G))
    return w, b


# ---------------------------------------------------------------- bass build

def _build_nc(T_tiles, MT, B, O_alloc, OBIG):
    import concourse.mybir as mybir
    from concourse import bacc
    from concourse.masks import make_identity
    from concourse.tile import TileContext

    dt = mybir.dt
    AF = mybir.ActivationFunctionType

    nc = bacc.Bacc()
    x_d = nc.dram_tensor("x", [T_tiles * P, H], dt.bfloat16, kind="ExternalInput")
    w_d = nc.dram_tensor("w", [4, 4, P, G], dt.bfloat16, kind="ExternalInput")
    b_d = nc.dram_tensor("b", [4, P, G], dt.float32, kind="ExternalInput")
    o_d = nc.dram_tensor("hc", [OBIG, H], dt.float32, kind="ExternalOutput")

    with TileContext(nc) as tc:
        with (
            tc.tile_pool(name="const", bufs=1) as constp,
            tc.tile_pool(name="aT", bufs=3) as aTp,
            tc.tile_pool(name="gsb", bufs=2) as gsbp,
            tc.tile_pool(name="acts", bufs=3) as actp,
            tc.tile_pool(name="hc", bufs=2) as hcp,
            tc.tile_pool(name="psg", bufs=2, space="PSUM") as psgp,
            tc.tile_pool(name="pst", bufs=2, space="PSUM") as pstp,
        ):
            w_sb = constp.tile([P, 16 * G], dt.bfloat16)
            for li in range(4):
                for k in range(4):
                    j = li * 4 + k
                    nc.gpsimd.dma_start(w_sb[:, j * G:(j + 1) * G], w_d[li, k])
            b_sb = constp.tile([P, 4 * G], dt.float32)
            for li in range(4):
                nc.gpsimd.dma_start(b_sb[:, li * G:(li + 1) * G], b_d[li])
            id_bf = constp.tile([P, P], dt.bfloat16)
            make_identity(nc, id_bf[:])
            id_f32 = constp.tile([P, P], dt.float32)
            make_identity(nc, id_f32[:])
            # all of x resident: row t*128+p -> partition p, cols [t*H,(t+1)*H)
            x_all = constp.tile([P, T_tiles * H], dt.bfloat16)
            nc.gpsimd.dma_start(
                x_all[:].rearrange("p (t h) -> p t h", h=H),
                x_d[:].rearrange("(t p) h -> p t h", p=P))

            for t in range(T_tiles):
                x_t = x_all[:, t * H:(t + 1) * H]

                pt = pstp.tile([P, H], dt.bfloat16, tag="pst")
                for k in range(4):
                    nc.tensor.transpose(pt[:, k * P:(k + 1) * P],
                                        x_t[:, k * P:(k + 1) * P], id_bf[:])
                aT = aTp.tile([P, H], dt.bfloat16, tag="aT")
                nc.vector.tensor_copy(aT[:], pt[:])

                hc_all = hcp.tile([P, J * H], dt.float32, tag="hc")
                for li in range(4):
                    h_t = hc_all[:, (2 * li) * H:(2 * li + 1) * H]
                    c_t = hc_all[:, (2 * li + 1) * H:(2 * li + 2) * H]
                    g_ps = psgp.tile([P, G], dt.float32, tag="psg")
                    for k in range(4):
                        lhsT = aT[:, k * P:(k + 1) * P]
                        wbase = (li * 4 + k) * G
                        for n in range(3):
                            nc.tensor.matmul(
                                g_ps[:, n * H:(n + 1) * H],
                                lhsT,
                                w_sb[:, wbase + n * H:wbase + (n + 1) * H],
                                start=(k == 0),
                                stop=(k == 3),
                            )
                    g_sb = gsbp.tile([P, G], dt.float32, tag="gsb")
                    nc.vector.tensor_add(g_sb[:], g_ps[:],
                                         b_sb[:, li * G:(li + 1) * G])
                    sio = actp.tile([P, 1024], dt.float32, tag="sio")
                    nc.scalar.activation(sio[:], g_sb[:, 0:1024], AF.Sigmoid)
                    tg = actp.tile([P, H], dt.float32, tag="tg")
                    nc.scalar.activation(tg[:], g_sb[:, 1024:G], AF.Tanh)
                    nc.vector.tensor_mul(c_t, sio[:, 0:H], tg[:])
                    tc_t = actp.tile([P, H], dt.float32, tag="tc")
                    nc.scalar.activation(tc_t[:], c_t, AF.Tanh)
                    nc.vector.tensor_mul(h_t, sio[:, H:1024], tc_t[:])

                    if li < 3:
                        pt2 = pstp.tile([P, H], dt.float32, tag="pst")
                        for k in range(4):
                            nc.tensor.transpose(pt2[:, k * P:(k + 1) * P],
                                                h_t[:, k * P:(k + 1) * P],
                                                id_f32[:])
                        aT = aTp.tile([P, H], dt.bfloat16, tag="aT")
                        nc.vector.tensor_copy(aT[:], pt2[:])

                # contiguous stores: slot k writes prefix rows [0, m) of the
                # tile for each of the 8 fused outputs
                for k, m in enumerate(MT[t]):
                    base = B[t][k]
                    for j in range(J):
                        nc.sync.dma_start(
                            o_d[j * O_alloc + base:j * O_alloc + base + m, :],
                            hc_all[0:m, j * H:(j + 1) * H])
    nc.compile()
    return nc


# ---------------------------------------------------------------- entry point

def _ensure_axon_hooks():
    """bass_utils' trace path imports antenv.axon_hooks, which some images
    lack; install a shim that drives NTFF profiling via libaxon_pjrt.so
    (mirrors the boot-side _ntff_profile_via_ctypes) or degrades to None."""
    try:
        import antenv.axon_hooks  # noqa: F401
        return
    except ImportError:
        pass
    import types
    import contextlib
    import ctypes

    def _build_hook():
        so = "/opt/axon/libaxon_pjrt.so"
        try:
            lib = ctypes.CDLL(so)
        except OSError:
            return None
        if not hasattr(lib, "axon_start_nrt_profile"):
            return None
        lib.axon_start_nrt_profile.argtypes = [
            ctypes.POINTER(ctypes.c_int64), ctypes.c_size_t]
        lib.axon_start_nrt_profile.restype = ctypes.c_int64
        lib.axon_stop_nrt_profile.argtypes = [ctypes.c_char_p]
        lib.axon_stop_nrt_profile.restype = ctypes.c_int64

        @contextlib.contextmanager
        def _hook(output_dir, device_ids):
            import jax
            jax.devices()
            if device_ids:
                ids = (ctypes.c_int64 * len(device_ids))(*device_ids)
                rc = lib.axon_start_nrt_profile(ids, len(device_ids))
            else:
                rc = lib.axon_start_nrt_profile(None, 0)
            if rc != 0:
                raise RuntimeError(f"axon_start_nrt_profile rc={rc}")
            try:
                yield
            finally:
                n = lib.axon_stop_nrt_profile(str(output_dir).encode())
                print(f"ntff profile: {n} file(s) written to {output_dir}",
                      file=sys.stderr)

        return _hook

    box = [None, False]

    def set_axon_ntff_profile_hook(h):
        box[0] = h
        box[1] = True

    def get_axon_ntff_profile_hook():
        if not box[1]:
            box[0] = _build_hook()
            box[1] = True
        return box[0]

    mod = types.ModuleType("antenv.axon_hooks")
    mod.set_axon_ntff_profile_hook = set_axon_ntff_profile_hook
    mod.get_axon_ntff_profile_hook = get_axon_ntff_profile_hook
    import antenv
    sys.modules["antenv.axon_hooks"] = mod
    antenv.axon_hooks = mod


_cache = {}


def kernel(**inputs):
    packed_x = np.asarray(inputs["packed_x"], np.float32)
    bs = np.asarray(inputs["batch_sizes"])

    key = bs.tobytes()
    if key not in _cache:
        plan = _make_plan(bs)
        nc = _build_nc(plan["T_tiles"], plan["MT"], plan["B"],
                       plan["O_alloc"], plan["OBIG"])
        _cache[key] = (plan, nc)
    plan, nc = _cache[key]

    w, b = _pack_weights(inputs)
    T_tiles = plan["T_tiles"]

    in_maps = []
    for cc in plan["cores"]:
        x = np.zeros((T_tiles * P, H), ml_dtypes.bfloat16)
        x[:len(cc["src_o"])] = packed_x[cc["src_o"]].astype(ml_dtypes.bfloat16)
        in_maps.append({"x": x, "w": w, "b": b})

    from concourse.bass_utils import run_bass_kernel_spmd
    _ensure_axon_hooks()
    res = run_bass_kernel_spmd(nc, in_maps, core_ids=list(range(NCORES)))
    global last_result
    last_result = res

    O_alloc = plan["O_alloc"]
    full = {}
    for jo, nm in enumerate(OUT_NAMES):
        f = np.zeros((plan["Nout"], H), np.float32)
        for c, cc in enumerate(plan["cores"]):
            slab = np.asarray(res.results[c]["hc"])[jo * O_alloc:(jo + 1) * O_alloc]
            f[cc["glob_rows"]] = slab[cc["slab_rows"]]
        full[nm] = f

    return (full["h4"], full["h1"], full["c1"], full["h2"], full["c2"],
            full["h3"], full["c3"], full["h4"], full["c4"])


if __name__ == "__main__":
    import reference
    inputs = reference.setup_inputs()
    out = kernel(**{k: np.asarray(v) for k, v in inputs.items()})
    print([o.shape for o in out])

